# revision 67
# baseline (speedup 1.0000x reference)
"""Causal group-query attention on 8 Trainium2 NeuronCores (fp16, v13).

Sharding: core c -> (batch b = c // 4, kv-group g = c % 4).
Each core owns batch element b, q-heads [4g, 4g+4) and kv-group g (n_rep = 4,
so those 4 q-heads attend to exactly kv-group g's k/v).  Every core computes
its partial o_proj output (contracting head-concat columns [512g, 512g+512)),
and the host sums the 4 partials per batch element (the "all-reduce after
o_proj" done host-side since we return full outputs anyway).

v13 vs the 289.5us v2 baseline (~255us, PE ~88% busy; trace-driven):
  * Boundary stalls: sweeps are ordered [q0,q1], [k,v], [q2,q3] ([k] first
    at tb=0) and bias/RoPE emission mirrors the sweep completion order, so
    q0/q1 RoPE finishes DURING phase A and phase B's first S matmuls have
    no boundary dependency (v2 head-blocked the ACT queue on the v-bias,
    serializing a ~2.4us bias->RoPE chain at every block boundary).
  * Softmax normalization (den/bc matmuls + reciprocal) of each head-pair
    is DEFERRED into the next pair's loop (i==min(8,imax)) or the next
    block's phase A, so the in-order PE queue never waits on the
    DVE/GpSimd acc-chain drain.
  * o_proj filler units are distributed evenly over BOTH pairs' iterations
    (14 in-loop + 2 at the boundary); near pair tails their evacs avoid
    ACT so the diagonal exps that gate the acc chain aren't delayed.
  * ACT runs only biases + exp + half the evacs; GpSimd cannot touch PSUM
    (all PSUM evacuations live on ACT/DVE; GpSimd keeps the odd-head acc
    chains).
  * DMA: x and Wq/Wk/Wv ship host-pre-swizzled into their SBUF image
    layouts so every load is a contiguous [128, N] copy with 4-16KB
    descriptors (natural layouts give 256B-1KB descriptors, which are
    latency-bound and poisoned the startup stream).  Everything rides the
    sync queue (hardware DGE; the gpsimd software DGE has ~4us startup,
    lower bandwidth, and a ~7us drain at kernel end), interleaved with the
    x tiles in need order.  Biases pack into one [128,6] tensor, mask and
    identity into one [128,256].
  * Output staging: the 4 o_proj tiles of a 128-row block share one
    [128, D] SBUF tile and go out as a single DMA (4KB descriptors, 4x
    fewer sync-queue issues; the issue cost ~0.65us each was the tail).

Per-core kernel (T=2048, D=2048, HS=128):
  phase A (per 512-wide t-block): stream x, compute Q^T/K^T/V^T fp16
    projections on the PE (contract over D in 16 k-tiles), add bias on ACT,
    RoPE on DVE as (cos*q + nsin*halfswap(q)); V^T transposed back to
    [t, hs] tiles via PE transpose (ACT/DVE evac).
  phase B (same t-block as q-block jq), heads in pairs: for each causally
    valid 128-wide k-tile i and head h: S^T = k-tile x q^T (fp16),
    P^T = exp(S^T/sqrt(HS) - 4) on ACT (fp16 out), causal mask folded into
    the diagonal subtile via a -400 matmul (LDWEIGHTS is double-buffered,
    so the extra matmul is nearly free); O^T += V @ P^T in PSUM;
    denominator acc += P on DVE (even heads) / GpSimd (odd heads), one
    ones-matmul partition-reduce per (block, head),
    reciprocal_approx_fast, K=1 broadcast matmul, multiply into O^T while
    evacuating (fp16 o_proj operand).
  phase C: o_proj partial out[tq, d] = sum_h O^T_h @ Wo^T_h, one t-block
    late so the PE fills the boundary + exp-latency gaps of the next block.
"""

import math

import numpy as np

B, T, D = 2, 2048, 2048
N_HEAD, N_GROUP = 16, 4
HS = D // N_HEAD  # 128
N_REP = N_HEAD // N_GROUP  # 4
NH_C = N_HEAD // N_GROUP  # heads per core = 4
INV_SQRT_HS = 1.0 / math.sqrt(HS)
EXP_SHIFT = -4.0  # exp(s*scale + shift): keeps fp16 P in range, cancels in softmax

_NC_CACHE: dict = {}


def build_nc(t=T):
    """Build and compile the per-core Bass program. Returns the compiled nc."""
    import concourse.bass as bass  # noqa: F401
    import concourse.mybir as mybir
    import concourse.tile as tile
    from concourse import bacc

    f32 = mybir.dt.float32
    f16 = mybir.dt.float16
    ident_f = mybir.ActivationFunctionType.Identity
    exp_f = mybir.ActivationFunctionType.Exp

    nd = D // 128  # d-tiles (contraction) = 16
    tb_n = t // 512  # 512-wide t blocks
    nk = t // 128  # 128-wide k tiles

    nc = bacc.Bacc("TRN2", target_bir_lowering=False, debug=False)

    # x ships pre-arranged t-block-major: row block tb holds [128, nd*512]
    # with 4KB-contiguous (d-tile, t) runs, so x DMAs get 4KB descriptors
    # instead of the 1KB the natural [D, t] layout allows
    xd = nc.dram_tensor("x_i", [(t // 512) * 128, nd * 512], f16,
                        kind="ExternalInput")
    # projection weights ship pre-swizzled into their SBUF image layout
    # [128, nd, m] so every DMA is a contiguous [128, N] copy with 4-16KB
    # descriptors (the natural [D, m] layout gives 256B-1KB descriptors,
    # which are latency-bound and poison the startup stream)
    wqd = nc.dram_tensor("wq_i", [128, nd * NH_C * HS], f16, kind="ExternalInput")
    wkd = nc.dram_tensor("wk_i", [128, nd * HS], f16, kind="ExternalInput")
    wvd = nc.dram_tensor("wv_i", [128, nd * HS], f16, kind="ExternalInput")
    wod = nc.dram_tensor("wo_t", [NH_C * HS, D], f16, kind="ExternalInput")
    cosd = nc.dram_tensor("cos_t", [HS, t], f16, kind="ExternalInput")
    # packed biases: cols 0..3 = bq per head, 4 = bk, 5 = bv (one DMA issue)
    biasd = nc.dram_tensor("bias_all", [HS, NH_C + 2], f32, kind="ExternalInput")
    nsind = nc.dram_tensor("nsin_t", [HS, t], f16, kind="ExternalInput")
    # packed [mask | identity] (one DMA issue)
    mid_d = nc.dram_tensor("maskid_t", [128, 256], f16, kind="ExternalInput")
    outd = nc.dram_tensor("out", [t, D], f16, kind="ExternalOutput")

    with tile.TileContext(nc) as tc:
        with (
            tc.tile_pool(name="consts", bufs=1) as consts,
            tc.tile_pool(name="wpool", bufs=1) as wpool,
            tc.tile_pool(name="resid", bufs=1) as resid,
            tc.tile_pool(name="xin", bufs=8) as xin,
            tc.tile_pool(name="work", bufs=3) as work,
            tc.tile_pool(name="qfp", bufs=2) as qfp,
            tc.tile_pool(name="otp", bufs=6) as otp,
            tc.tile_pool(name="outp", bufs=4) as outp,
            tc.tile_pool(name="psum", bufs=7, space="PSUM") as psum,
        ):
            def bank(name):
                return psum.tile([128, 512], f32, tag="bank", name=name)

            # ---- constants / weights (loaded once) ----
            cos_sb = consts.tile([128, t], f16, name="cos_sb")
            nsin_sb = consts.tile([128, t], f16, name="nsin_sb")
            mid_sb = consts.tile([128, 256], f16, name="mid_sb")
            maskb_sb = mid_sb[:, 0:128]
            id_sb = mid_sb[:, 128:256]
            ones_sb = consts.tile([128, 128], f16, name="ones_sb")
            bias_sb = consts.tile([128, NH_C + 2], f32, name="bias_sb")
            bq_sb = bias_sb[:, 0:NH_C]
            bk_sb = bias_sb[:, NH_C : NH_C + 1]
            bv_sb = bias_sb[:, NH_C + 1 : NH_C + 2]
            shift_sb = consts.tile([128, 1], f32, name="shift_sb")
            wq_sb = wpool.tile([128, nd, NH_C * HS], f16, name="wq_sb")
            wk_sb = wpool.tile([128, nd, HS], f16, name="wk_sb")
            wv_sb = wpool.tile([128, nd, HS], f16, name="wv_sb")
            wo_sb = wpool.tile([128, NH_C, D], f16, name="wo_sb")
            wq_re = wqd[:, :].rearrange("p (n m) -> p n m", n=nd)
            wk_re = wkd[:, :].rearrange("p (n m) -> p n m", n=nd)
            wv_re = wvd[:, :].rearrange("p (n m) -> p n m", n=nd)

            # resident K^T [hs, t] and V [t(128-tiles), hs]
            kt_sb = resid.tile([128, t], f16, name="kt_sb")
            v_sb = resid.tile([128, nk, HS], f16, name="v_sb")

            x_re = xd[:, :].rearrange("(b p) (n j) -> b p n j", p=128, j=512)

            ob_row = [None]

            def emit_oproj_unit(tb, ot_sb, s, db, evac=None):
                # one o_proj output tile for q-block tb; these are emitted one
                # block late, interleaved into the next block's attention loop
                # as PE filler so exp jitter on ACT never stalls the PE.
                # The 4 db tiles of a row-block share one [128, D] staging
                # tile and go out as a single DMA (4KB descriptors, 4x fewer
                # issues on the sync engine).
                ts0 = tb * 512
                op_ps = bank("op_ps")
                for h in range(NH_C):
                    nc.tensor.matmul(
                        op_ps,
                        lhsT=ot_sb[h][:, 128 * s : 128 * (s + 1)],
                        rhs=wo_sb[:, h, 512 * db : 512 * (db + 1)],
                        start=h == 0, stop=h == NH_C - 1,
                    )
                if db == 0:
                    ob_row[0] = outp.tile([128, D], f16, name="ob", bufs=3)
                ob = ob_row[0]
                # GpSimd cannot read PSUM; evacs alternate ACT/DVE
                if evac == "scalar" or (evac is None and (s + db) % 2 == 1):
                    nc.scalar.copy(out=ob[:, 512 * db : 512 * (db + 1)], in_=op_ps)
                else:
                    nc.vector.tensor_copy(
                        out=ob[:, 512 * db : 512 * (db + 1)], in_=op_ps
                    )
                if db == D // 512 - 1:
                    nc.sync.dma_start(
                        out=outd[ts0 + 128 * s : ts0 + 128 * (s + 1), :],
                        in_=ob,
                    )

            def oproj_units(tb, ot_sb):
                for s in range(4):
                    for db in range(D // 512):
                        yield (tb, ot_sb, s, db)

            def emit_norm(heads, ot_ps, acc, dst):
                # normalize each head's O^T by its softmax denominator:
                # one ones-matmul partition-reduces acc, fast-reciprocal
                # of the [1,512] PSUM row on DVE, broadcast the reciprocal
                # across partitions with a K=1 fp16 ones-matmul, and
                # multiply into O^T (DVE) while evacuating, written as the
                # fp16 o_proj operand.  Deferred past independent PE work
                # so the den matmul never stalls on the acc-chain drain.
                osb_h = {}
                rec_h = {}
                for h in heads:
                    den_ps = psum.tile(
                        [128, 512], f32, tag="bc", name="den_ps", bufs=1
                    )
                    nc.tensor.matmul(
                        den_ps[0:1, :],
                        lhsT=ones_sb[:, 0:1],
                        rhs=acc[h],
                        start=True, stop=True,
                    )
                    osb = otp.tile([128, 512], f32, name="osb", bufs=8)
                    nc.vector.tensor_copy(out=osb, in_=ot_ps[h])
                    osb_h[h] = osb
                    rec = work.tile([1, 512], f32, name="rec_sb", bufs=2)
                    nc.vector.reciprocal_approx_fast(
                        out=rec, in_=den_ps[0:1, :]
                    )
                    rec16 = work.tile([1, 512], f16, name="rec16", bufs=2)
                    nc.vector.tensor_copy(out=rec16, in_=rec)
                    rec_h[h] = rec16
                for h in heads:
                    bc_ps = psum.tile(
                        [128, 512], f32, tag="bc", name="bc_ps", bufs=1
                    )
                    nc.tensor.matmul(
                        bc_ps,
                        lhsT=ones_sb[0:1, 0:128],
                        rhs=rec_h[h],
                        start=True, stop=True,
                    )
                    osb16 = otp.tile([128, 512], f16, name="osb16", bufs=8)
                    nc.vector.tensor_mul(osb16, osb_h[h], bc_ps)
                    dst[h] = osb16

            pending_oproj = None
            pend_norm = None
            for tb in range(tb_n):
                ts0 = tb * 512
                # ============ phase A: projections + RoPE for this t-block
                qt_ps = [bank(f"qt_ps{h}") for h in range(NH_C)]
                kt_ps = bank("kt_ps")
                vt_ps = bank("vt_ps")
                # sweep order is chosen so the early outputs' bias + RoPE
                # run on ACT/DVE during the later sweeps, leaving phase B's
                # first S matmuls with no boundary dependency.  q2/q3 are
                # always last (their bias lands at the boundary, RoPE well
                # before head-pair (2,3) runs).  At tb=0, k goes alone in
                # sweep 0 so the first matmul only needs x^T + Wk from the
                # sync queue while Wq/Wv stream on the gpsimd queue.
                if tb == 0:
                    # k alone in sweep 0: the first matmuls need only
                    # x^T + Wk (1MB) from the sync queue, and the later
                    # sweeps' weights stream in behind the x tiles.  All
                    # loads ride the sync queue (hardware DGE: fast,
                    # parallel across 16 engines; the gpsimd software DGE
                    # has ~4us startup, lower bandwidth, and a ~7us drain
                    # at kernel end), in need order.
                    sweeps = [["k"], ["q0", "q1", "v"], ["q2", "q3"]]
                    nc.vector.memset(ones_sb, 1.0)
                    nc.vector.memset(shift_sb, EXP_SHIFT)
                else:
                    sweeps = [["q0", "q1"], ["k", "v"], ["q2", "q3"]]
                xts = []
                for sw, outs in enumerate(sweeps):
                    for chunk in range(nd // 4):
                        c4 = 4 * chunk
                        if sw == 0:
                            xt = xin.tile([128, 4, 512], f16, name="xt")
                            nc.sync.dma_start(
                                out=xt,
                                in_=x_re[tb, :, c4 : c4 + 4, :],
                            )
                            xts.append(xt)
                            if tb == 0 and chunk == 0:
                                nc.sync.dma_start(
                                    out=wk_sb[:, :, :], in_=wk_re[:, :, :]
                                )
                                nc.sync.dma_start(out=bias_sb, in_=biasd[:, :])
                            elif tb == 0 and chunk == 2:
                                nc.sync.dma_start(
                                    out=wq_sb[:, 0:4, :], in_=wq_re[:, 0:4, :]
                                )
                            elif tb == 0 and chunk == 3:
                                nc.sync.dma_start(
                                    out=wv_sb[:, :, :], in_=wv_re[:, :, :]
                                )
                                nc.sync.dma_start(
                                    out=cos_sb[:, 0:512], in_=cosd[:, 0:512]
                                )
                                nc.sync.dma_start(
                                    out=nsin_sb[:, 0:512], in_=nsind[:, 0:512]
                                )
                                nc.sync.dma_start(
                                    out=wq_sb[:, 4:8, :], in_=wq_re[:, 4:8, :]
                                )
                                nc.sync.dma_start(
                                    out=wq_sb[:, 8:12, :], in_=wq_re[:, 8:12, :]
                                )
                                nc.sync.dma_start(
                                    out=wq_sb[:, 12:nd, :],
                                    in_=wq_re[:, 12:nd, :],
                                )
                        if sw == 0 and chunk == 1 and pend_norm is not None:
                            # previous block's pair-(2,3) normalization:
                            # its den/bc PE matmuls slot in behind a tile of
                            # projection work, by which time the acc chains
                            # have drained -- no PE stall
                            emit_norm(*pend_norm)
                            pend_norm = None
                        xt = xts[chunk]
                        for j in range(4):
                            dt = c4 + j
                            first, last = dt == 0, dt == nd - 1
                            for o in outs:
                                if o == "k":
                                    nc.tensor.matmul(
                                        kt_ps, lhsT=wk_sb[:, dt, :],
                                        rhs=xt[:, j, :],
                                        start=first, stop=last,
                                    )
                                elif o == "v":
                                    nc.tensor.matmul(
                                        vt_ps, lhsT=wv_sb[:, dt, :],
                                        rhs=xt[:, j, :],
                                        start=first, stop=last,
                                    )
                                else:
                                    h = int(o[1])
                                    nc.tensor.matmul(
                                        qt_ps[h],
                                        lhsT=wq_sb[:, dt, h * HS : (h + 1) * HS],
                                        rhs=xt[:, j, :],
                                        start=first, stop=last,
                                    )

                if tb == 0:
                    # bulk loads that are only needed from phase B / tb=1 on
                    nc.sync.dma_start(out=mid_sb, in_=mid_d[:, :])
                    if t > 512:
                        nc.sync.dma_start(
                            out=cos_sb[:, 512:t], in_=cosd[:, 512:t]
                        )
                        nc.sync.dma_start(
                            out=nsin_sb[:, 512:t], in_=nsind[:, 512:t]
                        )
                    wo_re = wod[:, :].rearrange("(h p) m -> p h m", p=128)
                    for h in range(NH_C):
                        nc.sync.dma_start(
                            out=wo_sb[:, h : h + 1, :],
                            in_=wo_re[:, h : h + 1, :],
                        )

                # RoPE entirely on DVE: rot(q) is a half-swap across the
                # partition axis with the sign folded into the nsin constant
                qf = qfp.tile([128, NH_C, 512], f16, name="qf")

                def rope_q(h):
                    qraw = work.tile([128, 512], f16, name="qraw")
                    nc.scalar.activation(
                        out=qraw, in_=qt_ps[h], func=ident_f,
                        bias=bq_sb[:, h : h + 1], scale=1.0,
                    )
                    rtmp = work.tile([128, 512], f16, name="rtmp", bufs=2)
                    nc.vector.tensor_mul(
                        rtmp[0:64, :], qraw[64:128, :],
                        nsin_sb[64:128, ts0 : ts0 + 512],
                    )
                    nc.vector.tensor_mul(
                        rtmp[64:128, :], qraw[0:64, :],
                        nsin_sb[0:64, ts0 : ts0 + 512],
                    )
                    nc.vector.tensor_mul(
                        qf[:, h, :], qraw, cos_sb[:, ts0 : ts0 + 512]
                    )
                    nc.vector.tensor_add(qf[:, h, :], qf[:, h, :], rtmp)

                def rope_k():
                    # k: bias + rope -> kt_sb slice (fp16)
                    kraw = work.tile([128, 512], f16, name="qraw")
                    nc.scalar.activation(
                        out=kraw, in_=kt_ps, func=ident_f, bias=bk_sb[:, 0:1],
                        scale=1.0,
                    )
                    rtmp = work.tile([128, 512], f16, name="rtmp", bufs=2)
                    nc.vector.tensor_mul(
                        rtmp[0:64, :], kraw[64:128, :],
                        nsin_sb[64:128, ts0 : ts0 + 512],
                    )
                    nc.vector.tensor_mul(
                        rtmp[64:128, :], kraw[0:64, :],
                        nsin_sb[0:64, ts0 : ts0 + 512],
                    )
                    nc.vector.tensor_mul(
                        kt_sb[:, ts0 : ts0 + 512], kraw,
                        cos_sb[:, ts0 : ts0 + 512],
                    )
                    nc.vector.tensor_add(
                        kt_sb[:, ts0 : ts0 + 512], kt_sb[:, ts0 : ts0 + 512],
                        rtmp,
                    )

                # emission order mirrors the sweep completion order so no
                # engine queue head-blocks on a later-stopping PSUM source
                if tb == 0:
                    rope_k()
                    rope_q(0)
                    rope_q(1)
                else:
                    rope_q(0)
                    rope_q(1)
                    rope_k()

                # v bias on ACT (vt_ps stops at sweep 1 end)
                vraw = work.tile([128, 512], f16, name="vraw")
                nc.scalar.activation(
                    out=vraw, in_=vt_ps, func=ident_f, bias=bv_sb[:, 0:1], scale=1.0
                )

                # q2/q3 bias + RoPE last: their ACT biases land right at the
                # boundary (behind kraw/vraw which drained during sweep 2),
                # so the first exps queue just after them and their PSUM
                # banks free before phase B's st pipeline needs the space;
                # their DVE RoPE completes long before head-pair (2,3) runs.
                rope_q(2)
                rope_q(3)

                # previous block's o_proj units trickle into this block's
                # attention loop as PE filler
                op_iter = (
                    iter(oproj_units(tb - 1, pending_oproj))
                    if pending_oproj is not None else iter(())
                )
                # boundary filler: PE work while the q2/q3 biases and first
                # exps queue on ACT.  Only 2 units here (their PSUM banks
                # drain via DVE behind the q2/q3 RoPE, so more would pile up
                # in the 7-slot bank ring); the rest interleave in the loop.
                for bf in range(2):
                    u = next(op_iter, None)
                    if u is not None:
                        emit_oproj_unit(*u, evac="vector" if bf % 2 == 0 else "scalar")

                # transpose v to [t, hs] tiles (fp16); ALL evacs on ACT:
                # a DVE evac here makes the next o_proj filler's PSUM slot
                # wait behind exp-gated acc adds in the in-order DVE queue
                # (GpSimd cannot read PSUM)
                for s in range(4):
                    vt_tp = psum.tile([128, 512], f16, tag="bank", name="vt_tp")
                    nc.tensor.transpose(
                        vt_tp[:, 0:128], vraw[:, 128 * s : 128 * (s + 1)], id_sb[:, :]
                    )
                    nc.scalar.copy(
                        out=v_sb[:, 4 * tb + s, :], in_=vt_tp[:, 0:128]
                    )

                # ============ phase B: attention for q-block jq == tb
                # Software-pipelined by one k-tile: PV of tile i-1 issues
                # while ACT computes exp of tile i, so the PE never waits on
                # the st->exp->mask chain.  The softmax denominator is NOT a
                # per-tile PE matmul: P-tiles are accumulated element-wise
                # (acc += P_i) alternating DVE/GpSimd by (i+h) parity, and a
                # single ones-matmul per (q-block, head) reduces acc across
                # partitions -- 8x fewer denominator rows on the PE.
                ot_sb = {}
                imax = 4 * tb + 3
                # distribute the 14 in-loop o_proj filler units evenly over
                # BOTH pairs' iterations (one-per-iteration exhausts them in
                # pair (0,1), leaving pair (2,3)'s PE exp-paced at late tb)
                iters_total = 2 * (imax + 1)
                op_consumed = 2  # boundary fillers
                for hp in range(NH_C // 2):
                    heads = (2 * hp, 2 * hp + 1)
                    ot_ps = {h: bank(f"ot_ps{h}") for h in heads}
                    acc = {
                        h: qfp.tile([128, 512], f16, name=f"acc{h % 2}", bufs=2)
                        for h in heads
                    }

                    def emit_pv(i, pts, c0):
                        first, last = i == 0, i == imax
                        for h in heads:
                            nc.tensor.matmul(
                                ot_ps[h][:, c0:],
                                lhsT=v_sb[:, i, :],
                                rhs=pts[h][:, c0:],
                                start=first, stop=last,
                            )

                    pipeline = []
                    pt0 = None
                    for i in range(imax + 1):
                        c0 = 128 * max(0, i - 4 * tb)
                        diag = i >= 4 * tb
                        pts = {}
                        for h in heads:
                            st_ps = bank("st_ps")
                            if diag:
                                # causal mask folded into the score matmul:
                                # adds -400 to the strictly-lower (tk > tq)
                                # triangle of the diagonal 128-subtile; exp
                                # then underflows those to 0.  The extra
                                # matmul is ~free: LDWEIGHTS is hidden by
                                # double-buffering and N=128 costs 53ns.
                                nc.tensor.matmul(
                                    st_ps[:, c0 : c0 + 128],
                                    lhsT=kt_sb[:, 128 * i : 128 * (i + 1)],
                                    rhs=qf[:, h, c0 : c0 + 128],
                                    start=True, stop=False,
                                )
                                nc.tensor.matmul(
                                    st_ps[:, c0 : c0 + 128],
                                    lhsT=maskb_sb,
                                    rhs=id_sb,
                                    start=False, stop=True,
                                )
                                if c0 + 128 < 512:
                                    nc.tensor.matmul(
                                        st_ps[:, c0 + 128 :],
                                        lhsT=kt_sb[:, 128 * i : 128 * (i + 1)],
                                        rhs=qf[:, h, c0 + 128 :],
                                        start=True, stop=True,
                                    )
                            else:
                                nc.tensor.matmul(
                                    st_ps[:, c0:],
                                    lhsT=kt_sb[:, 128 * i : 128 * (i + 1)],
                                    rhs=qf[:, h, c0:],
                                    start=True, stop=True,
                                )
                            pt = work.tile([128, 512], f16, name="pt", bufs=8)
                            nc.scalar.activation(
                                out=pt[:, c0:], in_=st_ps[:, c0:], func=exp_f,
                                scale=INV_SQRT_HS, bias=shift_sb[:, 0:1],
                            )
                            pts[h] = pt
                        pipeline.append((i, pts, c0))
                        if len(pipeline) > 3:
                            emit_pv(*pipeline.pop(0))
                        idx = hp * (imax + 1) + i
                        target = 2 + ((idx + 1) * 14) // iters_total
                        while op_consumed < target:
                            u = next(op_iter, None)
                            if u is None:
                                break
                            # near the pair tail, keep evacs off ACT: a
                            # scalar COPY there delays the diagonal exps
                            # that gate the acc chain and the next den
                            emit_oproj_unit(
                                *u, evac="vector" if i > imax - 4 else None
                            )
                            op_consumed += 1
                        # previous pair's normalization: deferred until this
                        # pair's pipeline is rolling, so its den matmul sits
                        # behind independent PE work while the acc chains
                        # drain on DVE/GpSimd
                        if i == min(8, imax) and pend_norm is not None:
                            emit_norm(*pend_norm)
                            pend_norm = None
                        # accumulate P-tiles for the softmax denominator on
                        # DVE (even heads) / GpSimd (odd heads); each head's
                        # chain stays on one engine (cross-engine handoffs
                        # would block the in-order queues on semaphores)
                        if i == 1:
                            for h in heads:
                                eng = nc.vector if h % 2 == 0 else nc.gpsimd
                                eng.tensor_add(
                                    acc[h][:, c0:], pt0[h][:, c0:],
                                    pts[h][:, c0:],
                                )
                                if c0 > 0:
                                    eng.tensor_copy(
                                        out=acc[h][:, 0:c0],
                                        in_=pt0[h][:, 0:c0],
                                    )
                        elif i > 1:
                            for h in heads:
                                eng = nc.vector if h % 2 == 0 else nc.gpsimd
                                eng.tensor_add(
                                    acc[h][:, c0:], acc[h][:, c0:],
                                    pts[h][:, c0:],
                                )
                        if i == 0:
                            pt0 = pts
                    for rem in pipeline:
                        emit_pv(*rem)
                    pend_norm = (heads, ot_ps, acc, ot_sb)

                for u in op_iter:
                    emit_oproj_unit(*u)
                pending_oproj = ot_sb

            # final block: normalize pair (2,3), then its o_proj units
            emit_norm(*pend_norm)
            pend_norm = None
            for u in oproj_units(tb_n - 1, pending_oproj):
                emit_oproj_unit(*u)

    nc.compile()
    return nc


def shard_inputs(x, cos, sin, Wq, bq, Wkv, bkv, Wo, t=T):
    """Build the 8 per-core input maps (core c -> batch c//4, group c%4)."""
    f16 = np.float16
    f32 = np.float32
    hs = HS
    sin_t_f = sin.T.astype(np.float32)
    nsin_t = np.ascontiguousarray(
        np.concatenate([sin_t_f[hs // 2 :], -sin_t_f[: hs // 2]], 0).astype(f16)
    )
    # -400 on the strictly-lower (tk > tq) triangle, transposed for lhsT,
    # packed side by side with the 128x128 identity (single DMA)
    maskb_t = (-400.0 * np.tril(np.ones((128, 128), np.float32), -1)).T
    maskid_t = np.ascontiguousarray(
        np.concatenate([maskb_t, np.eye(128)], axis=1).astype(f16)
    )
    cos_t = np.ascontiguousarray(cos.T.astype(f16))

    # x image: [tb, p, n, j] = x.T[n*128+p, tb*512+j], flattened 2D
    tbn, ndd = t // 512, x.shape[2] // 128
    xts = [
        np.ascontiguousarray(
            x[b].T.astype(f16)
            .reshape(ndd, 128, tbn, 512)
            .transpose(2, 1, 0, 3)
            .reshape(tbn * 128, ndd * 512)
        )
        for b in range(x.shape[0])
    ]
    def swizzle(w_t):
        # [D, m] -> SBUF image [128, nd*m]: partition p holds d-tile-major
        # rows, so the device DMA is a contiguous [128, N] copy
        dd, m = w_t.shape
        return np.ascontiguousarray(
            w_t.reshape(dd // 128, 128, m).transpose(1, 0, 2).reshape(128, -1)
            .astype(f16)
        )

    per_g = []
    for g in range(4):
        # packed biases: cols 0..3 = bq per head, 4 = bk, 5 = bv
        bias_all = np.concatenate(
            [
                bq[512 * g : 512 * g + 512].reshape(4, 128).T,
                bkv[128 * g : 128 * g + 128].reshape(128, 1),
                bkv[512 + 128 * g : 512 + 128 * g + 128].reshape(128, 1),
            ],
            axis=1,
        )
        per_g.append(
            dict(
                wq_i=swizzle(Wq[512 * g : 512 * g + 512].T),
                bias_all=np.ascontiguousarray(bias_all.astype(f32)),
                wk_i=swizzle(Wkv[128 * g : 128 * g + 128].T),
                wv_i=swizzle(Wkv[512 + 128 * g : 512 + 128 * g + 128].T),
                wo_t=np.ascontiguousarray(
                    Wo[:, 512 * g : 512 * g + 512].T.astype(f16)
                ),
            )
        )

    in_maps = []
    for c in range(4 * x.shape[0]):
        b, g = c // 4, c % 4
        m = dict(per_g[g])
        m.update(
            x_i=xts[b], cos_t=cos_t,
            nsin_t=nsin_t, maskid_t=maskid_t,
        )
        in_maps.append(m)
    return in_maps


def run_on_hw(in_maps, t=T, trace=False, **flags):
    from concourse.bass_utils import run_bass_kernel_spmd

    key = (t, tuple(sorted(flags.items())))
    if key not in _NC_CACHE:
        _NC_CACHE[key] = build_nc(t, **flags)
    nc = _NC_CACHE[key]
    res = run_bass_kernel_spmd(
        nc, in_maps, core_ids=list(range(len(in_maps))), trace=trace
    )
    return res


def kernel(x, cos, sin, Wq, bq, Wkv, bkv, Wo):
    x = np.asarray(x)
    in_maps = shard_inputs(
        x, np.asarray(cos), np.asarray(sin), np.asarray(Wq), np.asarray(bq),
        np.asarray(Wkv), np.asarray(bkv), np.asarray(Wo),
    )
    res = run_on_hw(in_maps, t=T, trace=False)
    out = np.zeros((B, T, D), np.float32)
    for c, rmap in enumerate(res.results):
        out[c // 4] += rmap["out"].astype(np.float32)
    return out
